# revision 7
# baseline (speedup 1.0000x reference)
"""Trainium2 Bass kernel for nn_ClBlockLoss (contrastive block loss).

Reference math (z: [4, 2048, 128] f32):
    s  = z.reshape(8192, 128); sn = s / ||s||
    sim = (sn @ sn.T) / 0.5
    per row i (residue r = i % 2048, block q = i // 2048):
      negatives = cols j with j % 2048 != r
      positives = cols q'*2048 + r, q' != q   (3 of them)
      loss += sum_p [ log(exp(pos_p) + sum_neg exp(sim)) - pos_p ]
    loss /= 4*3*2048

Sharding: data-parallel over the 8192 rows; core c owns rows
[c*1024, (c+1)*1024). Each core receives a per-core *permuted* copy of
the full z (blocks rotated so its own block is block 0, residues rotated
so its rows are positions 0..1023). The permutation is a bijection, so
full-row sums are unchanged, and it makes the positives/self columns
land on compile-time-constant diagonals -> one SPMD program, zero
collectives. Since sim in [-2, 2], exp() cannot overflow, so no max
subtraction is needed: S0 = sum_all exp - sum_sameresidue exp.

Each core returns [128, 24] loss terms (partition p, (row_tile t, pos j));
the host sums all 8*128*24 terms in float64 and divides by 24576.
"""

import numpy as np

# Problem constants (hardcoded; kernel.py must be self-contained).
N_T = 4
BS = 2048
D = 128
K = N_T * BS          # 8192
TEMP = 0.5
KPOS = N_T * (N_T - 1) * BS  # 24576
N_CORES = 8
ROWS_PER_CORE = K // N_CORES  # 1024
P = 128
NT = K // P           # 64 row-tiles of the full matrix
NT_OWN = ROWS_PER_CORE // P  # 8 row-tiles owned per core
GRP = 8               # z row-tiles per DMA/compute group
NGRP = NT // GRP      # 8
CHUNK = 2048          # ACT exp chunk (4 PSUM banks)
NCHUNK = K // CHUNK   # 4
MM_N = 512            # matmul moving free dim (1 PSUM bank fp32)
E2 = float(np.exp(np.float32(2.0)))  # exp(self-sim), self cos == 1

_CACHE = {}


def _build_program():
    import concourse.bass as bass
    import concourse.tile as tile
    from concourse import bacc, masks, mybir

    f32 = mybir.dt.float32
    f32r = mybir.dt.float32r
    AF = mybir.ActivationFunctionType
    OP = mybir.AluOpType
    AX = mybir.AxisListType

    nc = bacc.Bacc(
        "TRN2",
        target_bir_lowering=False,
        debug=False,
        enable_asserts=False,
        num_devices=N_CORES,
    )

    zp = nc.dram_tensor("zp", [K, D], f32, kind="ExternalInput")
    out = nc.dram_tensor("partial", [P, NT_OWN * 3], f32, kind="ExternalOutput")

    # DRAM view: [p, u, d] with row = u*128 + p
    zp_r = zp.ap().rearrange("(u p) d -> p u d", p=P)

    with tile.TileContext(nc) as tc:
        with (
            tc.tile_pool(name="zrows", bufs=NGRP) as zrows_pool,
            tc.tile_pool(name="snt", bufs=NCHUNK) as snt_pool,
            tc.tile_pool(name="stats", bufs=1) as stats_pool,
            tc.tile_pool(name="scratch", bufs=3) as scratch_pool,
            tc.tile_pool(name="diag", bufs=8) as diag_pool,
            tc.tile_pool(name="psum", bufs=2, space="PSUM") as psum_pool,
        ):
            i32 = mybir.dt.int32
            identity = stats_pool.tile([P, P], f32)
            masks.make_identity(nc, identity[:])

            # dummy exp: preload the ACT Exp table set off the critical path
            dummy = stats_pool.tile([P, 1], f32)
            nc.scalar.activation(dummy[:], identity[:, 0:1], AF.Exp)

            ztiles = [
                zrows_pool.tile([P, GRP, D], f32, tag="zr", name=f"zr{g}")
                for g in range(NGRP)
            ]
            snt = [
                snt_pool.tile([P, CHUNK], f32r, tag="snt", name=f"snt{i}")
                for i in range(NCHUNK)
            ]

            sumsq = stats_pool.tile([P, NT], f32)
            inv = stats_pool.tile([P, NT], f32)
            sacc = stats_pool.tile([P, NT_OWN * NCHUNK], f32)

            for g in range(NGRP):
                for h in range(2):
                    nc.sync.dma_start(
                        ztiles[g][:, h * 4:(h + 1) * 4, :],
                        zp_r[:, g * GRP + h * 4:g * GRP + (h + 1) * 4, :],
                    )

            def emit_sumsq(g):
                for k in range(GRP):
                    u = g * GRP + k
                    sq_scr = scratch_pool.tile([P, D], f32, tag="sq", name=f"sq{u}")
                    nc.vector.scalar_tensor_tensor(
                        out=sq_scr[:], in0=ztiles[g][:, k, :], scalar=1.0,
                        in1=ztiles[g][:, k, :], op0=OP.bypass, op1=OP.mult,
                        accum_out=sumsq[:, u:u + 1],
                    )

            def emit_newton(p):
                # inv[:, 16p:16p+16] = rsqrt(sumsq) via bit-trick seed + 3 Newton
                cols = slice(16 * p, 16 * p + 16)
                x = sumsq[:, cols]
                y = inv[:, cols]
                sh = scratch_pool.tile([P, 16], f32, tag="nw_sh", name=f"nwsh{p}")
                t1 = scratch_pool.tile([P, 16], f32, tag="nw_t1", name=f"nwt1{p}")
                t2 = scratch_pool.tile([P, 16], f32, tag="nw_t2", name=f"nwt2{p}")
                nc.vector.tensor_scalar(
                    out=sh[:].bitcast(i32), in0=x.bitcast(i32), scalar1=1,
                    scalar2=None, op0=OP.arith_shift_right,
                )
                nc.vector.tensor_scalar(
                    out=y.bitcast(i32), in0=sh[:].bitcast(i32),
                    scalar1=0x5F3759DF, scalar2=-1, op0=OP.subtract, op1=OP.mult,
                )
                for _ in range(3):
                    nc.vector.tensor_mul(t1[:], y, y)
                    nc.vector.tensor_mul(t1[:], t1[:], x)
                    nc.vector.tensor_scalar(
                        out=t2[:], in0=t1[:], scalar1=-0.5, scalar2=1.5,
                        op0=OP.mult, op1=OP.add,
                    )
                    nc.vector.tensor_mul(y, y, t2[:])

            def emit_tbatch(p, b):
                # normalize+transpose 4 row-tiles into snt[p] via one psum batch
                bt = psum_pool.tile([P, 4 * P], f32, tag="mm", name=f"tp{p}_{b}")
                for k in range(4):
                    u = 16 * p + 4 * b + k
                    sn_scr = diag_pool.tile([P, P], f32, tag="dg", name=f"sn{u}")
                    nc.gpsimd.tensor_scalar_mul(
                        sn_scr[:], ztiles[u // GRP][:, u % GRP, :], inv[:, u:u + 1])
                    nc.tensor.transpose(
                        bt[:, k * P:(k + 1) * P], sn_scr[:], identity[:],
                    )
                nc.vector.tensor_copy(snt[p][:, b * 4 * P:(b + 1) * 4 * P], bt[:])

            def emit_chunk(t, ci):
                ps = psum_pool.tile([P, CHUNK], f32, tag="mm", name=f"c{t}_{ci}")
                lhsT = snt[0][:, t * P:(t + 1) * P]
                for nn in range(CHUNK // MM_N):
                    nc.tensor.matmul(
                        ps[:, nn * MM_N:(nn + 1) * MM_N], lhsT,
                        snt[ci][:, nn * MM_N:(nn + 1) * MM_N],
                        start=True, stop=True,
                    )
                nc.scalar.activation(
                    ps[:], ps[:], AF.Exp, scale=2.0,
                    accum_out=sacc[:, t * NCHUNK + ci:t * NCHUNK + ci + 1],
                )

            rawdot = stats_pool.tile([P, NT_OWN * 3], f32)
            cd = stats_pool.tile([P, NT_OWN * 3], f32)

            def emit_dots():
                for t in range(NT_OWN):
                    for j in range(3):
                        u2 = (j + 1) * 16 + t
                        dg_scr = scratch_pool.tile(
                            [P, D], f32, tag="sq", name=f"dot{t}_{j}")
                        nc.vector.scalar_tensor_tensor(
                            out=dg_scr[:], in0=ztiles[0][:, t, :], scalar=1.0,
                            in1=ztiles[u2 // GRP][:, u2 % GRP, :],
                            op0=OP.bypass, op1=OP.mult,
                            accum_out=rawdot[:, t * 3 + j:t * 3 + j + 1],
                        )
                    tcols = slice(t * 3, t * 3 + 3)
                    nc.vector.tensor_scalar_mul(
                        cd[:, tcols], rawdot[:, tcols], inv[:, t:t + 1]
                    )
                    inv3 = inv[:].rearrange("p (a b) -> p a b", b=16)[:, 1:4, t]
                    nc.vector.tensor_mul(cd[:, tcols], cd[:, tcols], inv3)

            # ---- prologue: panel 0 ready ASAP, rest of norms behind it ----
            emit_sumsq(0)
            emit_sumsq(1)
            emit_newton(0)
            for b in range(4):
                emit_tbatch(0, b)
            emit_sumsq(2)
            emit_sumsq(3)
            emit_newton(1)
            emit_sumsq(4)
            emit_sumsq(5)
            emit_newton(2)
            emit_sumsq(6)
            emit_sumsq(7)
            emit_newton(3)

            # ---- main: panel p chunks with panel p+1 prep woven in ----
            dots_done = False
            for p in range(NCHUNK):
                for t in range(NT_OWN):
                    emit_chunk(t, p)
                    if p + 1 < NCHUNK and t >= 4:
                        emit_tbatch(p + 1, t - 4)
                if not dots_done:
                    emit_dots()
                    dots_done = True

            # ---- final loss assembly ----
            e_all = stats_pool.tile([P, NT_OWN * 3], f32)
            nc.scalar.activation(e_all[:], cd[:], AF.Exp, scale=2.0)

            esum3 = stats_pool.tile([P, NT_OWN], f32)
            nc.vector.reduce_sum(
                esum3[:], e_all[:].rearrange("p (t j) -> p t j", j=3), axis=AX.X
            )
            stot = stats_pool.tile([P, NT_OWN], f32)
            nc.vector.reduce_sum(
                stot[:], sacc[:].rearrange("p (t c) -> p t c", c=NCHUNK), axis=AX.X
            )
            s0 = stats_pool.tile([P, NT_OWN], f32)
            nc.vector.tensor_sub(s0[:], stot[:], esum3[:])
            nc.vector.tensor_scalar_add(s0[:], s0[:], -E2)

            lin = stats_pool.tile([P, NT_OWN * 3], f32)
            for t in range(NT_OWN):
                tcols = slice(t * 3, t * 3 + 3)
                nc.vector.tensor_scalar_add(
                    lin[:, tcols], e_all[:, tcols], s0[:, t:t + 1]
                )
            logs = stats_pool.tile([P, NT_OWN * 3], f32)
            nc.scalar.activation(logs[:], lin[:], AF.Ln)

            terms = stats_pool.tile([P, NT_OWN * 3], f32)
            nc.vector.scalar_tensor_tensor(
                out=terms[:], in0=cd[:], scalar=-2.0, in1=logs[:],
                op0=OP.mult, op1=OP.add,
            )
            nc.sync.dma_start(out.ap(), terms[:])

    nc.compile()
    return nc


def _get_program():
    if "nc" not in _CACHE:
        _CACHE["nc"] = _build_program()
    return _CACHE["nc"]


def _permute_for_core(zf: np.ndarray, c: int) -> np.ndarray:
    """Rotate blocks/residues so core c's rows are positions [0, 1024)."""
    qc, half = divmod(c, 2)
    r0 = half * ROWS_PER_CORE
    u = np.arange(BS)
    perm = np.concatenate(
        [((qc + qp) % N_T) * BS + ((r0 + u) % BS) for qp in range(N_T)]
    )
    return np.ascontiguousarray(zf[perm])


def kernel(z: np.ndarray) -> np.ndarray:
    from concourse.bass_utils import run_bass_kernel_spmd

    zf = np.asarray(z, dtype=np.float32).reshape(K, D)
    nc = _get_program()
    in_maps = [{"zp": _permute_for_core(zf, c)} for c in range(N_CORES)]
    res = run_bass_kernel_spmd(
        nc, in_maps, core_ids=list(range(N_CORES)),
        trace=bool(int(__import__("os").environ.get("KERNEL_TRACE", "0"))),
    )
    if res.exec_time_ns is not None:
        _CACHE["exec_time_ns"] = res.exec_time_ns
    total = sum(
        float(np.sum(r["partial"].astype(np.float64))) for r in res.results
    )
    return np.float32(total / KPOS)
